# revision 1
# baseline (speedup 1.0000x reference)
"""Causal self-attention (B=2, T=2048, D=1024, H=16) on 8 TRN2 NeuronCores.

Sharding: 8-way tensor-parallel over heads (2 heads/core, both batches),
then one 8-core AllToAll reshards from head-channels to token-slices so each
core computes a disjoint [512, 1024] slice of the output projection.

Per-core program (SPMD, identical program, per-core data):
  core r: heads {2r, 2r+1}  -> qkv channel slice [128r : 128r+128)
          output slice      -> batch r//4, tokens [512*(r%4), 512*(r%4)+512)

bf16 matmul inputs (host-converted), fp32 PSUM accumulation, fp32 output.
Numpy-simulated end-to-end max rel err vs the fp32 reference: ~3.3e-3.

Attention is processed as (batch, 512-query-block) groups, two groups in
flight round-robin, with both heads' scores packed into one [128,1024] PSUM
tile so each k-block costs a single Exp on ScalarE. Causality is handled by
skipping above-diagonal k-blocks plus one additive -240 mask matmul on the
diagonal tile (exp -> ~0). The softmax denominator comes from an appended
ones-column in V'; normalization is reciprocal + GpSimd partition-broadcast.
"""

import numpy as np
import ml_dtypes
from contextlib import ExitStack

import concourse.bass as bass
import concourse.tile as tile
from concourse import mybir, bacc
from concourse.bass_utils import run_bass_kernel_spmd

F32 = mybir.dt.float32
BF16 = mybir.dt.bfloat16

B, T, D, H, HD = 2, 2048, 1024, 16, 64
NC = 8  # cores
TI = B * T  # token instances = 4096
SCALE = HD ** -0.5


def build_nc() -> bass.Bass:
    nc = bacc.Bacc("TRN2", target_bir_lowering=False, debug=False, num_devices=NC)

    xf = nc.dram_tensor("xf", [TI, D], BF16, kind="ExternalInput").ap()
    wq = nc.dram_tensor("wq", [D, 128], BF16, kind="ExternalInput").ap()
    wk = nc.dram_tensor("wk", [D, 128], BF16, kind="ExternalInput").ap()
    wv = nc.dram_tensor("wv", [D, 128], BF16, kind="ExternalInput").ap()
    bq = nc.dram_tensor("bq", [128], BF16, kind="ExternalInput").ap()
    bk = nc.dram_tensor("bk", [128], BF16, kind="ExternalInput").ap()
    bv = nc.dram_tensor("bv", [128], BF16, kind="ExternalInput").ap()
    wo = nc.dram_tensor("wo", [D, D], BF16, kind="ExternalInput").ap()
    bo = nc.dram_tensor("bo", [D], BF16, kind="ExternalInput").ap()
    # additive causal mask, pre-transposed for lhsT (0 on/below diag, -240 above)
    tri = nc.dram_tensor("tri", [128, 128], BF16, kind="ExternalInput").ap()
    eye = nc.dram_tensor("eye", [128, 128], BF16, kind="ExternalInput").ap()
    out = nc.dram_tensor("out", [512, D], F32, kind="ExternalOutput").ap()

    with tile.TileContext(nc) as tc, ExitStack() as ctx:
        const = ctx.enter_context(tc.tile_pool(name="const", bufs=1))
        qkvp = ctx.enter_context(tc.tile_pool(name="qkvp", bufs=1))
        xload = ctx.enter_context(tc.tile_pool(name="xload", bufs=5))
        xtp = ctx.enter_context(tc.tile_pool(name="xtp", bufs=2))
        vtb = ctx.enter_context(tc.tile_pool(name="vtb", bufs=2))
        ptp = ctx.enter_context(tc.tile_pool(name="ptp", bufs=6))
        rp = ctx.enter_context(tc.tile_pool(name="rp", bufs=2))
        atp = ctx.enter_context(tc.tile_pool(name="atp", bufs=3))
        aoutp = ctx.enter_context(tc.tile_pool(name="aoutp", bufs=1))
        osb = ctx.enter_context(tc.tile_pool(name="osb", bufs=2))
        psS = ctx.enter_context(tc.tile_pool(name="psS", bufs=2, space="PSUM"))
        psB = ctx.enter_context(tc.tile_pool(name="psB", bufs=4, space="PSUM"))
        dram = ctx.enter_context(tc.tile_pool(name="dram", bufs=1, space="DRAM"))

        # ---- constants / weights -------------------------------------------------
        wq_sb = const.tile([128, D], BF16)  # col 128c+m  <- wq[128c+p, m]
        wk_sb = const.tile([128, D], BF16)
        wv_sb = const.tile([128, D], BF16)
        nc.sync.dma_start(
            wq_sb[:].rearrange("p (c m) -> p c m", c=8),
            wq.rearrange("(c p) m -> p c m", p=128),
        )
        nc.sync.dma_start(
            wk_sb[:].rearrange("p (c m) -> p c m", c=8),
            wk.rearrange("(c p) m -> p c m", p=128),
        )
        nc.sync.dma_start(
            wv_sb[:].rearrange("p (c m) -> p c m", c=8),
            wv.rearrange("(c p) m -> p c m", p=128),
        )
        wo_sb = const.tile([128, 8 * D], BF16)  # col 1024c+n <- wo[128c+p, n]
        nc.sync.dma_start(
            wo_sb[:].rearrange("p (c n) -> p c n", c=8),
            wo.rearrange("(c p) n -> p c n", p=128),
        )
        bq_sb = const.tile([1, 128], BF16)
        bk_sb = const.tile([1, 128], BF16)
        bv_sb = const.tile([1, 128], BF16)
        bo_sb = const.tile([1, D], BF16)
        nc.sync.dma_start(bq_sb[:], bq[None, :])
        nc.sync.dma_start(bk_sb[:], bk[None, :])
        nc.sync.dma_start(bv_sb[:], bv[None, :])
        nc.sync.dma_start(bo_sb[:], bo[None, :])
        tri_sb = const.tile([128, 128], BF16)
        eye_sb = const.tile([128, 128], BF16)
        nc.sync.dma_start(tri_sb[:], tri[:])
        nc.sync.dma_start(eye_sb[:], eye[:])
        ones_sb = const.tile([1, 512], BF16)
        nc.vector.memset(ones_sb[:], 1.0)

        # Q^T | K^T packed: col t -> Q^T, col TI + t -> K^T  (channels on partitions)
        qkt_sb = qkvp.tile([128, 2 * TI], BF16)
        # V' : [kpos(128), 32 ktiles x (2 heads x 65)]; col 130*kt + 65*h + d,
        # d==64 is the ones column (softmax denominator trick)
        vp_sb = qkvp.tile([128, 32 * 130], BF16)
        vp_ones = vp_sb.rearrange("p (kt h d) -> p kt h d", kt=32, h=2, d=65)[
            :, :, :, 64:65
        ]
        nc.vector.memset(vp_ones, 1.0)

        a2a_in = dram.tile([1024, 512], BF16)
        a2a_out = dram.tile([1024, 512], BF16)

        # ---- phase A/B: x^T then QKV projections, per 512-token block.
        # Emitted as a generator of PE-sized chunks so batch 1's projection
        # work can be interleaved into batch 0's attention emission (keeps the
        # TensorEngine fed while ScalarE runs the exps; Tile's dependency
        # tracking preserves correctness regardless of emission order).
        def ab_block(b, blk):
            base = 2048 * b + 512 * blk
            xts = []
            for i in range(4):
                x_t = xload.tile([128, D], BF16, name="x_t")
                nc.sync.dma_start(x_t[:], xf[base + 128 * i : base + 128 * (i + 1), :])
                xts.append(x_t)
            xT = xtp.tile([128, 8 * 512], BF16)  # col 512c + t
            for c2 in range(4):
                pst = psS.tile([128, 1024], BF16, name="ps_t", tag="pss")
                for ci in range(2):
                    c = 2 * c2 + ci
                    for i in range(4):
                        nc.tensor.transpose(
                            pst[:, 512 * ci + 128 * i : 512 * ci + 128 * (i + 1)],
                            xts[i][:, 128 * c : 128 * (c + 1)],
                            eye_sb[:],
                        )
                nc.vector.tensor_copy(xT[:, 1024 * c2 : 1024 * (c2 + 1)], pst[:])
                yield

            # Q^T and K^T share one 2-bank psum tile; single strided evacuation
            qk = psS.tile([128, 1024], F32, name="ps_qk", tag="pss")
            for half, (w_sb, b_sb) in enumerate(((wq_sb, bq_sb), (wk_sb, bk_sb))):
                sl = slice(512 * half, 512 * (half + 1))
                for c in range(8):
                    nc.tensor.matmul(
                        qk[:, sl],
                        w_sb[:, 128 * c : 128 * (c + 1)],
                        xT[:, 512 * c : 512 * (c + 1)],
                        start=(c == 0),
                        stop=False,
                    )
                nc.tensor.matmul(
                    qk[:, sl], b_sb[:], ones_sb[:], start=False, stop=True
                )
                yield
            qk_dst = qkt_sb[:].rearrange("p (s t) -> p s t", s=2)[
                :, :, base : base + 512
            ]
            nc.scalar.copy(qk_dst, qk.rearrange("p (s t) -> p s t", s=2))

            # V^T then V' tiles via PE transpose
            vps = psS.tile([128, 512], F32, name="ps_v", tag="pss")
            for c in range(8):
                nc.tensor.matmul(
                    vps[:],
                    wv_sb[:, 128 * c : 128 * (c + 1)],
                    xT[:, 512 * c : 512 * (c + 1)],
                    start=(c == 0),
                    stop=False,
                )
            nc.tensor.matmul(
                vps[:], bv_sb[:], ones_sb[:], start=False, stop=True
            )
            vt_blk = vtb.tile([128, 512], BF16)
            nc.vector.tensor_copy(vt_blk[:], vps[:])
            yield
            ps2 = psS.tile([128, 512], BF16, name="ps_vt", tag="pss")
            for i in range(4):
                nc.tensor.transpose(
                    ps2[:, 128 * i : 128 * (i + 1)],
                    vt_blk[:, 128 * i : 128 * (i + 1)],
                    eye_sb[:],
                )
            kt0 = 16 * b + 4 * blk
            dst = vp_sb[:, 130 * kt0 : 130 * (kt0 + 4)].rearrange(
                "p (kt h d) -> p kt h d", kt=4, h=2, d=65
            )[:, :, :, :64]
            src = ps2.rearrange("p (i h d) -> p i h d", i=4, h=2, d=64)
            nc.vector.tensor_copy(dst, src)
            yield

        # batch 0 projections emitted up front; batch 1 paced into phase C
        for blk in range(4):
            for _ in ab_block(0, blk):
                pass
        import itertools
        ab1 = itertools.chain.from_iterable(ab_block(1, blk) for blk in range(4))

        # ---- phase C: attention; two (batch, q-block) groups in flight -----------
        def kq(sl_base, lo, hi):
            return qkt_sb[:, sl_base + lo : sl_base + hi]

        class Group:
            def __init__(self, b, j):
                self.b, self.j = b, j
                self.nkb = 4 * j + 4
                self.kb_s = 0  # next k-block to score
                self.kb_a = 0  # next k-block to accumulate into AV
                self.qbase = 2048 * b + 512 * j
                self.avs = [
                    psB.tile([128, 512], F32, name=f"av{h}", tag="av")
                    for h in range(2)
                ]
                self.pts = {}

            def emit_scores(self):
                kb = self.kb_s
                self.kb_s += 1
                m = kb - 4 * self.j
                off = 128 * m if m >= 0 else 0
                ps_s = psS.tile([128, 1024], F32, name="ps_s", tag="pss")
                for h in range(2):
                    hr = 64 * h
                    hb = 512 * h
                    nc.tensor.matmul(
                        ps_s[:, hb + off : hb + 512],
                        qkt_sb[
                            hr : hr + 64,
                            TI + 2048 * self.b + 128 * kb : TI + 2048 * self.b + 128 * (kb + 1),
                        ],
                        qkt_sb[hr : hr + 64, self.qbase + off : self.qbase + 512],
                        start=True,
                        stop=(m < 0),
                    )
                    if m >= 0:
                        # additive causal mask: ps += tri[qi, ki] (-240 above diag)
                        nc.tensor.matmul(
                            ps_s[:, hb + off : hb + off + 128],
                            tri_sb[:],
                            eye_sb[:],
                            start=False,
                            stop=True,
                        )
                pt = ptp.tile([128, 1024], BF16, name="pt")
                pt_v = pt.rearrange("p (s t) -> p s t", s=2)[:, :, off:512]
                ps_v = ps_s.rearrange("p (s t) -> p s t", s=2)[:, :, off:512]
                nc.scalar.activation(
                    pt_v, ps_v, mybir.ActivationFunctionType.Exp, scale=SCALE
                )
                self.pts[kb] = (pt, off)

            def emit_av(self):
                kb = self.kb_a
                self.kb_a += 1
                pt, off = self.pts.pop(kb)
                for h in range(2):
                    nc.tensor.matmul(
                        self.avs[h][0:65, off:],
                        vp_sb[
                            :,
                            130 * (16 * self.b + kb) + 65 * h : 130 * (16 * self.b + kb) + 65 * h + 65,
                        ],
                        pt[:, 512 * h + off : 512 * (h + 1)],
                        start=(kb == 0),
                        stop=(kb == self.nkb - 1),
                    )

            def finalize(self):
                s = 4 * self.b + self.j  # destination core (token-slice owner)
                for h in range(2):
                    rec = rp.tile([1, 512], F32, name="rec")
                    nc.vector.reciprocal(rec[:], self.avs[h][64:65, :])
                    rbc = rp.tile([64, 512], F32, name="rbc")
                    nc.gpsimd.partition_broadcast(rbc[:], rec[:])
                    at = atp.tile([64, 512], BF16, name="at")
                    nc.vector.tensor_mul(at[:], self.avs[h][0:64, :], rbc[:])
                    nc.sync.dma_start(
                        a2a_in[128 * s + 64 * h : 128 * s + 64 * h + 64, :], at[:]
                    )

        # long groups first so two groups stay in flight most of the time
        queue = [(0, 3), (0, 2), (0, 1), (0, 0), (1, 3), (1, 2), (1, 1), (1, 0)]
        active = []
        drained = [False]

        def pace_ab1(n=1):
            if drained[0]:
                return
            for _ in range(n):
                if next(ab1, "end") == "end":
                    drained[0] = True
                    return

        while queue or active:
            while len(active) < 2 and queue:
                if queue[0][0] == 1:
                    pace_ab1(1000)  # batch-1 group: its inputs must be emitted
                g = Group(*queue.pop(0))
                g.emit_scores()
                active.append(g)
            for g in list(active):
                if g.kb_s < g.nkb:
                    g.emit_scores()
                pace_ab1(1)
                # AV trails scores by 2 k-blocks so the exp always has slack
                # before the PE consumes it (drain once scores are exhausted)
                if g.kb_a < g.kb_s - 1 or (g.kb_s == g.nkb and g.kb_a < g.nkb):
                    g.emit_av()
                if g.kb_a == g.nkb:
                    g.finalize()
                    active.remove(g)
        pace_ab1(1000)

        # ---- phase D: reshard heads->tokens --------------------------------------
        nc.gpsimd.collective_compute(
            "AllToAll",
            mybir.AluOpType.bypass,
            replica_groups=[list(range(NC))],
            ins=[a2a_in.opt()],
            outs=[a2a_out.opt()],
        )

        # ---- phase E: output projection for my 512-token slice -------------------
        attn2 = aoutp.tile([128, 8 * 512], BF16)  # col 512c + t  (= attn^T chunks)
        for c in range(8):
            nc.sync.dma_start(
                attn2[:, 512 * c : 512 * (c + 1)],
                a2a_out[128 * c : 128 * (c + 1), :],
            )
        for mt in range(4):
            po = psS.tile([128, 1024], F32, name="ps_o", tag="pss")
            for nh in range(2):
                sl = slice(512 * nh, 512 * (nh + 1))
                for c in range(8):
                    nc.tensor.matmul(
                        po[:, sl],
                        attn2[:, 512 * c + 128 * mt : 512 * c + 128 * (mt + 1)],
                        wo_sb[:, 1024 * c + 512 * nh : 1024 * c + 512 * (nh + 1)],
                        start=(c == 0),
                        stop=False,
                    )
                nc.tensor.matmul(
                    po[:, sl],
                    ones_sb[:, 0:128],
                    bo_sb[:, 512 * nh : 512 * (nh + 1)],
                    start=False,
                    stop=True,
                )
            o_t = osb.tile([128, D], F32, name="o_t")
            nc.vector.tensor_copy(o_t[:], po[:])
            nc.sync.dma_start(out[128 * mt : 128 * (mt + 1), :], o_t[:])

    nc.compile()
    return nc


_NC_CACHE = None


def _get_nc():
    global _NC_CACHE
    if _NC_CACHE is None:
        _NC_CACHE = build_nc()
    return _NC_CACHE


def _b16(a):
    return np.ascontiguousarray(np.asarray(a, np.float32).astype(ml_dtypes.bfloat16))


def make_in_maps(x, Wq, bq, Wk, bk, Wv, bv, Wo, bo):
    xf = _b16(np.asarray(x, np.float32).reshape(TI, D))
    Wq, Wk, Wv, Wo = _b16(Wq), _b16(Wk), _b16(Wv), _b16(Wo)
    bq, bk, bv, bo = _b16(bq), _b16(bk), _b16(bv), _b16(bo)
    # additive causal mask, passed pre-transposed for lhsT:
    # want ps[ki, qi] += A[ki, qi], A = 0 if ki <= qi else -240;
    # matmul adds lhsT[qi, ki] so send A^T
    A = np.where(np.arange(128)[:, None] <= np.arange(128)[None, :], 0.0, -240.0)
    tri = np.ascontiguousarray(A.T.astype(ml_dtypes.bfloat16))
    eye = np.eye(128, dtype=ml_dtypes.bfloat16)
    in_maps = []
    for r in range(NC):
        ch = slice(128 * r, 128 * (r + 1))
        in_maps.append(
            {
                "xf": xf,
                "wq": np.ascontiguousarray(Wq[:, ch]),
                "wk": np.ascontiguousarray(Wk[:, ch]),
                "wv": np.ascontiguousarray(Wv[:, ch]),
                "bq": np.ascontiguousarray(bq[ch]),
                "bk": np.ascontiguousarray(bk[ch]),
                "bv": np.ascontiguousarray(bv[ch]),
                "wo": Wo,
                "bo": bo,
                "tri": tri,
                "eye": eye,
            }
        )
    return in_maps


def assemble(results):
    out = np.empty((B, T, D), np.float32)
    for r in range(NC):
        out[r // 4, 512 * (r % 4) : 512 * (r % 4 + 1), :] = results[r]["out"]
    return out


def run(inputs, trace=False, **kw):
    nc = _get_nc()
    in_maps = make_in_maps(**inputs)
    res = run_bass_kernel_spmd(nc, in_maps, core_ids=list(range(NC)), trace=trace, **kw)
    return assemble(res.results), res


def kernel(**inputs) -> np.ndarray:
    out, _ = run(inputs)
    return out



# revision 4
# speedup vs baseline: 1.2245x; 1.2245x over previous
"""Causal self-attention (B=2, T=2048, D=1024, H=16) on 8 TRN2 NeuronCores.

Sharding: 8-way tensor-parallel over heads (2 heads/core, both batches).
The head->token reshard is split into TWO AllToAlls (batch 0, then batch 1)
so the first collective and half the output projection overlap with batch-1
attention compute; slice ownership is 256-token interleaved (core r owns
tokens [256r, 256r+256) of BOTH batches).

Per-core program (SPMD, identical program, per-core data):
  core r: heads {2r, 2r+1}  -> qkv channel slice [128r : 128r+128)
          output slice      -> tokens [256r, 256r+256) of b0 and b1

x^T is pre-transposed on the host (removes all PE transposes of x), QKV
biases are folded into the PSUM evacuation (DVE tensor_scalar_add with a
per-partition bias), the causal mask is a 0/1 DVE multiply on the exp'd
probabilities (no mask matmuls), and softmax normalization uses
reciprocal_approx_fast (~5x faster than DVE reciprocal, ~18 bits).

bf16 matmul inputs, fp32 PSUM accumulation, fp32 output.

Attention is processed as (batch, 512-query-block) groups, two groups in
flight round-robin; per k-block each head's scores go to a 1-bank PSUM tile
(pss tag, 3 bufs) and one Exp per head on ScalarE. Causality is handled by
skipping above-diagonal k-blocks plus the 0/1 mask mul on the diagonal tile.
The softmax denominator comes from an appended ones-column in V';
normalization is reciprocal + GpSimd partition-broadcast.

PSUM budget: pss 3x1 bank + av (attention accumulators) 4x1 bank +
po (out-proj, dedicated tag to avoid PE-FIFO/slot-release deadlocks) 1 bank.
"""

import numpy as np
import ml_dtypes
import itertools
from contextlib import ExitStack

import concourse.bass as bass
import concourse.tile as tile
from concourse import mybir, bacc
from concourse.bass_utils import run_bass_kernel_spmd

F32 = mybir.dt.float32
BF16 = mybir.dt.bfloat16

B, T, D, H, HD = 2, 2048, 1024, 16, 64
NC = 8  # cores
TI = B * T  # token instances = 4096
SCALE = HD ** -0.5


def build_nc() -> bass.Bass:
    nc = bacc.Bacc("TRN2", target_bir_lowering=False, debug=False, num_devices=NC)

    xt = nc.dram_tensor("xt", [D, TI], BF16, kind="ExternalInput").ap()
    wq = nc.dram_tensor("wq", [D, 128], BF16, kind="ExternalInput").ap()
    wk = nc.dram_tensor("wk", [D, 128], BF16, kind="ExternalInput").ap()
    wv = nc.dram_tensor("wv", [D, 128], BF16, kind="ExternalInput").ap()
    bq = nc.dram_tensor("bq", [128, 1], F32, kind="ExternalInput").ap()
    bk = nc.dram_tensor("bk", [128, 1], F32, kind="ExternalInput").ap()
    bv = nc.dram_tensor("bv", [128, 1], F32, kind="ExternalInput").ap()
    wo = nc.dram_tensor("wo", [D, D], BF16, kind="ExternalInput").ap()
    bo = nc.dram_tensor("bo", [D], BF16, kind="ExternalInput").ap()
    # 0/1 causal keep-mask (1 where k<=q), duplicated horizontally for 2 heads
    tri2 = nc.dram_tensor("tri2", [128, 256], BF16, kind="ExternalInput").ap()
    eye = nc.dram_tensor("eye", [128, 128], BF16, kind="ExternalInput").ap()
    out = nc.dram_tensor("out", [512, D], F32, kind="ExternalOutput").ap()

    with tile.TileContext(nc) as tc, ExitStack() as ctx:
        const = ctx.enter_context(tc.tile_pool(name="const", bufs=1))
        qkvp = ctx.enter_context(tc.tile_pool(name="qkvp", bufs=1))
        xtp = ctx.enter_context(tc.tile_pool(name="xtp", bufs=1))
        vtb = ctx.enter_context(tc.tile_pool(name="vtb", bufs=2))
        ptp = ctx.enter_context(tc.tile_pool(name="ptp", bufs=6))
        rp = ctx.enter_context(tc.tile_pool(name="rp", bufs=2))
        atp = ctx.enter_context(tc.tile_pool(name="atp", bufs=3))
        aoutp = ctx.enter_context(tc.tile_pool(name="aoutp", bufs=2))
        osb = ctx.enter_context(tc.tile_pool(name="osb", bufs=2))
        psS = ctx.enter_context(tc.tile_pool(name="psS", bufs=3, space="PSUM"))
        psB = ctx.enter_context(tc.tile_pool(name="psB", bufs=4, space="PSUM"))
        dram = ctx.enter_context(tc.tile_pool(name="dram", bufs=1, space="DRAM"))

        # ---- constants / weights -------------------------------------------------
        wq_sb = const.tile([128, D], BF16)  # col 128c+m  <- wq[128c+p, m]
        wk_sb = const.tile([128, D], BF16)
        wv_sb = const.tile([128, D], BF16)
        nc.sync.dma_start(
            wq_sb[:].rearrange("p (c m) -> p c m", c=8),
            wq.rearrange("(c p) m -> p c m", p=128),
        )
        nc.sync.dma_start(
            wk_sb[:].rearrange("p (c m) -> p c m", c=8),
            wk.rearrange("(c p) m -> p c m", p=128),
        )
        nc.sync.dma_start(
            wv_sb[:].rearrange("p (c m) -> p c m", c=8),
            wv.rearrange("(c p) m -> p c m", p=128),
        )
        wo_sb = const.tile([128, 8 * D], BF16)  # col 1024c+n <- wo[128c+p, n]
        nc.sync.dma_start(
            wo_sb[:].rearrange("p (c n) -> p c n", c=8),
            wo.rearrange("(c p) n -> p c n", p=128),
        )
        bq_sb = const.tile([128, 1], F32)
        bk_sb = const.tile([128, 1], F32)
        bv_sb = const.tile([128, 1], F32)
        bo_sb = const.tile([1, D], BF16)
        nc.sync.dma_start(bq_sb[:], bq[:])
        nc.sync.dma_start(bk_sb[:], bk[:])
        nc.sync.dma_start(bv_sb[:], bv[:])
        nc.sync.dma_start(bo_sb[:], bo[None, :])
        tri2_sb = const.tile([128, 256], BF16)
        eye_sb = const.tile([128, 128], BF16)
        nc.sync.dma_start(tri2_sb[:], tri2[:])
        nc.sync.dma_start(eye_sb[:], eye[:])
        ones_sb = const.tile([1, 512], BF16)
        nc.vector.memset(ones_sb[:], 1.0)

        # x^T staged in SBUF: col 4096c + t  <- xT[128c+p, t]
        xt_sb = xtp.tile([128, 8 * TI], BF16)
        for blk8 in range(8):
            tb = 512 * blk8
            nc.sync.dma_start(
                xt_sb[:].rearrange("p (c t) -> p c t", c=8)[:, :, tb : tb + 512],
                xt.rearrange("(c p) t -> p c t", p=128)[:, :, tb : tb + 512],
            )

        # Q^T | K^T packed: col t -> Q^T, col TI + t -> K^T  (channels on partitions)
        qkt_sb = qkvp.tile([128, 2 * TI], BF16)
        # V' : [kpos(128), 32 ktiles x (2 heads x 65)]; col 130*kt + 65*h + d,
        # d==64 is the ones column (softmax denominator trick)
        vp_sb = qkvp.tile([128, 32 * 130], BF16)
        vp_ones = vp_sb.rearrange("p (kt h d) -> p kt h d", kt=32, h=2, d=65)[
            :, :, :, 64:65
        ]
        nc.vector.memset(vp_ones, 1.0)

        # two half-sized AllToAlls: b=0 slices, then b=1 slices.
        # rows 256*j + 128*sl + 64*h + p  (slice s=2j+sl -> dest core s)
        a2a_in = [dram.tile([1024, 256], BF16, name=f"a2a_in{b}") for b in range(2)]
        a2a_out = [dram.tile([1024, 256], BF16, name=f"a2a_out{b}") for b in range(2)]

        # ---- phase A/B: QKV projections, per 512-token block.
        # Generator of PE-sized chunks so batch 1's projection work can be
        # interleaved into batch 0's attention emission.
        def ab_block(b, blk):
            base = 2048 * b + 512 * blk

            def proj(w_sb, b_sb, dst):
                ps = psS.tile([128, 512], F32, name="ps_p", tag="pss")
                for c in range(8):
                    nc.tensor.matmul(
                        ps[:],
                        w_sb[:, 128 * c : 128 * (c + 1)],
                        xt_sb[:, 4096 * c + base : 4096 * c + base + 512],
                        start=(c == 0),
                        stop=(c == 7),
                    )
                # evacuate with bias folded in (per-partition scalar add)
                nc.vector.tensor_scalar_add(dst, ps[:], b_sb[:])

            proj(wq_sb, bq_sb, qkt_sb[:, base : base + 512])
            yield
            proj(wk_sb, bk_sb, qkt_sb[:, TI + base : TI + base + 512])
            yield
            vt_blk = vtb.tile([128, 512], BF16, name="vt_blk")
            proj(wv_sb, bv_sb, vt_blk[:])
            yield
            # V' tiles via PE transpose
            ps2 = psS.tile([128, 512], BF16, name="ps_vt", tag="pss")
            for i in range(4):
                nc.tensor.transpose(
                    ps2[:, 128 * i : 128 * (i + 1)],
                    vt_blk[:, 128 * i : 128 * (i + 1)],
                    eye_sb[:],
                )
            kt0 = 16 * b + 4 * blk
            dst = vp_sb[:, 130 * kt0 : 130 * (kt0 + 4)].rearrange(
                "p (kt h d) -> p kt h d", kt=4, h=2, d=65
            )[:, :, :, :64]
            src = ps2[:].rearrange("p (i h d) -> p i h d", i=4, h=2, d=64)
            nc.vector.tensor_copy(dst, src)
            yield

        # batch 0 projections emitted up front; batch 1 paced into phase C
        for blk in range(4):
            for _ in ab_block(0, blk):
                pass
        ab1 = itertools.chain.from_iterable(ab_block(1, blk) for blk in range(4))

        # ---- out-projection pass for one 256-token half (after a2a b) ------------
        def outproj_pass(b):
            attn2 = aoutp.tile([128, 8 * 256], BF16, name="attn2")  # col 256c+t
            for c in range(8):
                nc.sync.dma_start(
                    attn2[:, 256 * c : 256 * (c + 1)],
                    a2a_out[b][128 * c : 128 * (c + 1), :],
                )
            for mt in range(2):
                for nh in range(2):
                    sl = slice(512 * nh, 512 * (nh + 1))
                    po = psB.tile([128, 512], F32, name="ps_o", tag="po", bufs=1)
                    for c in range(8):
                        nc.tensor.matmul(
                            po[:],
                            attn2[:, 256 * c + 128 * mt : 256 * c + 128 * (mt + 1)],
                            wo_sb[:, 1024 * c + 512 * nh : 1024 * c + 512 * (nh + 1)],
                            start=(c == 0),
                            stop=False,
                        )
                    nc.tensor.matmul(
                        po[:], ones_sb[:, 0:128], bo_sb[:, sl], start=False, stop=True
                    )
                    o_t = osb.tile([128, 512], F32, name="o_t")
                    nc.vector.tensor_copy(o_t[:], po[:])
                    nc.sync.dma_start(
                        out[256 * b + 128 * mt : 256 * b + 128 * (mt + 1), sl], o_t[:]
                    )
                    yield

        # ---- phase C: attention; two (batch, q-block) groups in flight -----------
        class Group:
            def __init__(self, b, j):
                self.b, self.j = b, j
                self.nkb = 4 * j + 4
                self.kb_s = 0  # next k-block to score
                self.kb_a = 0  # next k-block to accumulate into AV
                self.qbase = 2048 * b + 512 * j
                self.avs = [
                    psB.tile([128, 512], F32, name=f"av{h}", tag="av")
                    for h in range(2)
                ]
                self.pts = {}

            def emit_scores(self):
                kb = self.kb_s
                self.kb_s += 1
                m = kb - 4 * self.j
                off = 128 * m if m >= 0 else 0
                kbase = TI + 2048 * self.b + 128 * kb
                pt = ptp.tile([128, 1024], BF16, name="pt")
                for h in range(2):
                    hr = 64 * h
                    ps_s = psS.tile([128, 512], F32, name="ps_s", tag="pss")
                    nc.tensor.matmul(
                        ps_s[:, off:512],
                        qkt_sb[hr : hr + 64, kbase : kbase + 128],
                        qkt_sb[hr : hr + 64, self.qbase + off : self.qbase + 512],
                        start=True,
                        stop=True,
                    )
                    nc.scalar.activation(
                        pt[:, 512 * h + off : 512 * (h + 1)],
                        ps_s[:, off:512],
                        mybir.ActivationFunctionType.Exp,
                        scale=SCALE,
                    )
                if m >= 0:
                    # zero the strictly-upper triangle of the diagonal
                    # 128x128 tile (both heads in one DVE op)
                    ptd = pt.rearrange("p (s t) -> p s t", s=2)[:, :, off : off + 128]
                    nc.vector.tensor_mul(
                        ptd, ptd, tri2_sb[:].rearrange("p (s t) -> p s t", s=2)
                    )
                self.pts[kb] = (pt, off)

            def emit_av(self):
                kb = self.kb_a
                self.kb_a += 1
                pt, off = self.pts.pop(kb)
                vb = 130 * (16 * self.b + kb)
                for h in range(2):
                    nc.tensor.matmul(
                        self.avs[h][0:65, off:],
                        vp_sb[:, vb + 65 * h : vb + 65 * h + 65],
                        pt[:, 512 * h + off : 512 * (h + 1)],
                        start=(kb == 0),
                        stop=(kb == self.nkb - 1),
                    )

            def finalize(self):
                for h in range(2):
                    # custom-DVE ops drop the input partition offset: stage the
                    # den row to a partition-0 SBUF tile before the reciprocal
                    den = rp.tile([1, 512], F32, name="den")
                    nc.vector.tensor_copy(den[:], self.avs[h][64:65, :])
                    rec = rp.tile([1, 512], F32, name="rec")
                    nc.vector.reciprocal_approx_fast(rec[:], den[:])
                    rbc = rp.tile([64, 512], F32, name="rbc")
                    nc.gpsimd.partition_broadcast(rbc[:], rec[:])
                    at = atp.tile([64, 512], BF16, name="at")
                    nc.vector.tensor_mul(at[:], self.avs[h][0:64, :], rbc[:])
                    dst = a2a_in[self.b].rearrange(
                        "(j sl h p) q -> j h p sl q", j=4, sl=2, h=2, p=64
                    )[self.j, h]
                    nc.sync.dma_start(
                        dst, at[:].rearrange("p (sl q) -> p sl q", sl=2)
                    )

        def emit_a2a(b):
            nc.gpsimd.collective_compute(
                "AllToAll",
                mybir.AluOpType.bypass,
                replica_groups=[list(range(NC))],
                ins=[a2a_in[b].opt()],
                outs=[a2a_out[b].opt()],
            )

        # long groups first so two groups stay in flight most of the time
        queue = [(0, 3), (0, 2), (0, 1), (0, 0), (1, 3), (1, 2), (1, 1), (1, 0)]
        active = []
        state = {"ab1_done": False, "b0_left": 4, "op0": None, "op0_done": False}

        def pace_ab1(n=1):
            if state["ab1_done"]:
                return
            for _ in range(n):
                if next(ab1, "end") == "end":
                    state["ab1_done"] = True
                    return

        def pace_op0(n=1):
            if state["op0"] is None or state["op0_done"]:
                return
            for _ in range(n):
                if next(state["op0"], "end") == "end":
                    state["op0_done"] = True
                    return

        while queue or active:
            while len(active) < 2 and queue:
                if queue[0][0] == 1:
                    pace_ab1(1000)  # batch-1 group: its inputs must be emitted
                g = Group(*queue.pop(0))
                g.emit_scores()
                active.append(g)
            for g in list(active):
                if g.kb_s < g.nkb:
                    g.emit_scores()
                pace_ab1(1)
                pace_op0(1)
                # AV trails scores by 2 k-blocks so the exp always has slack
                # before the PE consumes it (drain once scores are exhausted)
                if g.kb_a < g.kb_s - 1 or (g.kb_s == g.nkb and g.kb_a < g.nkb):
                    g.emit_av()
                if g.kb_a == g.nkb:
                    g.finalize()
                    active.remove(g)
                    if g.b == 0:
                        state["b0_left"] -= 1
                        if state["b0_left"] == 0:
                            emit_a2a(0)  # overlaps batch-1 attention
                    elif state["op0"] is None:
                        # first b1 group done: a2a#1 has had ~25us; start the
                        # b0-half output projection, paced into the schedule
                        state["op0"] = outproj_pass(0)
        pace_ab1(1000)
        pace_op0(1000)

        # ---- tail: second reshard + b1-half output projection --------------------
        emit_a2a(1)
        for _ in outproj_pass(1):
            pass

    nc.compile()
    return nc


_NC_CACHE = None


def _get_nc():
    global _NC_CACHE
    if _NC_CACHE is None:
        _NC_CACHE = build_nc()
    return _NC_CACHE


def _b16(a):
    return np.ascontiguousarray(np.asarray(a, np.float32).astype(ml_dtypes.bfloat16))


def make_in_maps(x, Wq, bq, Wk, bk, Wv, bv, Wo, bo):
    xt = _b16(np.asarray(x, np.float32).reshape(TI, D).T)  # [D, TI]
    Wq, Wk, Wv, Wo = _b16(Wq), _b16(Wk), _b16(Wv), _b16(Wo)
    bo16 = _b16(bo)
    bqf = np.asarray(bq, np.float32).reshape(D, 1)
    bkf = np.asarray(bk, np.float32).reshape(D, 1)
    bvf = np.asarray(bv, np.float32).reshape(D, 1)
    # 0/1 keep mask (1 where k<=q), duplicated for both heads
    tri01 = np.where(np.arange(128)[:, None] <= np.arange(128)[None, :], 1.0, 0.0)
    tri2 = np.ascontiguousarray(
        np.concatenate([tri01, tri01], axis=1).astype(ml_dtypes.bfloat16)
    )
    eye = np.eye(128, dtype=ml_dtypes.bfloat16)
    in_maps = []
    for r in range(NC):
        ch = slice(128 * r, 128 * (r + 1))
        in_maps.append(
            {
                "xt": xt,
                "wq": np.ascontiguousarray(Wq[:, ch]),
                "wk": np.ascontiguousarray(Wk[:, ch]),
                "wv": np.ascontiguousarray(Wv[:, ch]),
                "bq": np.ascontiguousarray(bqf[ch]),
                "bk": np.ascontiguousarray(bkf[ch]),
                "bv": np.ascontiguousarray(bvf[ch]),
                "wo": Wo,
                "bo": bo16,
                "tri2": tri2,
                "eye": eye,
            }
        )
    return in_maps


def assemble(results):
    out = np.empty((B, T, D), np.float32)
    for r in range(NC):
        res = results[r]["out"]
        out[0, 256 * r : 256 * (r + 1), :] = res[0:256]
        out[1, 256 * r : 256 * (r + 1), :] = res[256:512]
    return out


def run(inputs, trace=False, **kw):
    nc = _get_nc()
    in_maps = make_in_maps(**inputs)
    res = run_bass_kernel_spmd(nc, in_maps, core_ids=list(range(NC)), trace=trace, **kw)
    return assemble(res.results), res


def kernel(**inputs) -> np.ndarray:
    out, _ = run(inputs)
    return out


# revision 5
# speedup vs baseline: 1.3001x; 1.0618x over previous
"""Causal self-attention (B=2, T=2048, D=1024, H=16) on 8 TRN2 NeuronCores.

Sharding: 8-way tensor-parallel over heads (2 heads/core, both batches).
The head->token reshard is split into TWO AllToAlls (batch 0, then batch 1)
so the first collective and half the output projection overlap with batch-1
attention compute; slice ownership is 256-token interleaved (core r owns
tokens [256r, 256r+256) of BOTH batches).

Per-core program (SPMD, identical program, per-core data):
  core r: heads {2r, 2r+1}  -> qkv channel slice [128r : 128r+128)
          output slice      -> tokens [256r, 256r+256) of b0 and b1

x^T is pre-transposed on the host (removes all PE transposes of x), QKV
biases are folded into the PSUM evacuation (DVE tensor_scalar_add with a
per-partition bias), the causal mask is a 0/1 DVE multiply on the exp'd
probabilities (no mask matmuls), and softmax normalization uses
reciprocal_approx_fast (~5x faster than DVE reciprocal, ~18 bits).

bf16 matmul inputs, fp32 PSUM accumulation, fp32 output.

Attention is processed as (batch, 512-query-block) groups, two groups in
flight round-robin; per k-block each head's scores go to a 1-bank PSUM tile
(pss tag, 3 bufs) and one Exp per head on ScalarE. Causality is handled by
skipping above-diagonal k-blocks plus the 0/1 mask mul on the diagonal tile.
The softmax denominator comes from an appended ones-column in V';
normalization is reciprocal + GpSimd partition-broadcast.

PSUM budget: pss 3x1 bank + av (attention accumulators) 4x1 bank +
po (out-proj, dedicated tag to avoid PE-FIFO/slot-release deadlocks) 1 bank.
"""

import numpy as np
import ml_dtypes
import itertools
from contextlib import ExitStack

import concourse.bass as bass
import concourse.tile as tile
from concourse import mybir, bacc
from concourse.bass_utils import run_bass_kernel_spmd

F32 = mybir.dt.float32
BF16 = mybir.dt.bfloat16

B, T, D, H, HD = 2, 2048, 1024, 16, 64
NC = 8  # cores
TI = B * T  # token instances = 4096
SCALE = HD ** -0.5


def build_nc() -> bass.Bass:
    nc = bacc.Bacc("TRN2", target_bir_lowering=False, debug=False, num_devices=NC)

    xt = nc.dram_tensor("xt", [D, TI], BF16, kind="ExternalInput").ap()
    wq = nc.dram_tensor("wq", [D, 128], BF16, kind="ExternalInput").ap()
    wk = nc.dram_tensor("wk", [D, 128], BF16, kind="ExternalInput").ap()
    wv = nc.dram_tensor("wv", [D, 128], BF16, kind="ExternalInput").ap()
    bq = nc.dram_tensor("bq", [128, 1], F32, kind="ExternalInput").ap()
    bk = nc.dram_tensor("bk", [128, 1], F32, kind="ExternalInput").ap()
    bv = nc.dram_tensor("bv", [128, 1], F32, kind="ExternalInput").ap()
    wo = nc.dram_tensor("wo", [D, D], BF16, kind="ExternalInput").ap()
    bo = nc.dram_tensor("bo", [D], BF16, kind="ExternalInput").ap()
    # 0/1 causal keep-mask (1 where k<=q), duplicated horizontally for 2 heads
    tri2 = nc.dram_tensor("tri2", [128, 256], BF16, kind="ExternalInput").ap()
    eye = nc.dram_tensor("eye", [128, 128], BF16, kind="ExternalInput").ap()
    out = nc.dram_tensor("out", [512, D], F32, kind="ExternalOutput").ap()

    with tile.TileContext(nc) as tc, ExitStack() as ctx:
        const = ctx.enter_context(tc.tile_pool(name="const", bufs=1))
        qkvp = ctx.enter_context(tc.tile_pool(name="qkvp", bufs=1))
        xtp = ctx.enter_context(tc.tile_pool(name="xtp", bufs=1))
        vtb = ctx.enter_context(tc.tile_pool(name="vtb", bufs=2))
        ptp = ctx.enter_context(tc.tile_pool(name="ptp", bufs=6))
        rp = ctx.enter_context(tc.tile_pool(name="rp", bufs=2))
        atp = ctx.enter_context(tc.tile_pool(name="atp", bufs=3))
        aoutp = ctx.enter_context(tc.tile_pool(name="aoutp", bufs=2))
        osb = ctx.enter_context(tc.tile_pool(name="osb", bufs=2))
        psS = ctx.enter_context(tc.tile_pool(name="psS", bufs=3, space="PSUM"))
        psB = ctx.enter_context(tc.tile_pool(name="psB", bufs=4, space="PSUM"))
        dram = ctx.enter_context(tc.tile_pool(name="dram", bufs=1, space="DRAM"))

        # ---- constants / weights -------------------------------------------------
        # DMA order matters: the PE's first work (batch-0 projections) needs
        # wq/wk/wv + xt blocks 0-3; everything else (wo especially, 2MB) waits.
        wq_sb = const.tile([128, D], BF16)  # col 128c+m  <- wq[128c+p, m]
        wk_sb = const.tile([128, D], BF16)
        wv_sb = const.tile([128, D], BF16)
        bq_sb = const.tile([128, 1], F32)
        bk_sb = const.tile([128, 1], F32)
        bv_sb = const.tile([128, 1], F32)
        bo_sb = const.tile([1, D], BF16)
        wo_sb = const.tile([128, 8 * D], BF16)  # col 1024c+n <- wo[128c+p, n]
        tri2_sb = const.tile([128, 256], BF16)
        eye_sb = const.tile([128, 128], BF16)
        ones_sb = const.tile([1, 512], BF16)
        xt_sb = xtp.tile([128, 8 * TI], BF16)  # col 4096c + t <- xT[128c+p, t]

        def load_xt_block(blk8):
            tb = 512 * blk8
            nc.sync.dma_start(
                xt_sb[:].rearrange("p (c t) -> p c t", c=8)[:, :, tb : tb + 512],
                xt.rearrange("(c p) t -> p c t", p=128)[:, :, tb : tb + 512],
            )

        for w_sb, w in ((wq_sb, wq), (wk_sb, wk), (wv_sb, wv)):
            nc.sync.dma_start(
                w_sb[:].rearrange("p (c m) -> p c m", c=8),
                w.rearrange("(c p) m -> p c m", p=128),
            )
        load_xt_block(0)
        nc.sync.dma_start(bq_sb[:], bq[:])
        nc.sync.dma_start(bk_sb[:], bk[:])
        nc.sync.dma_start(bv_sb[:], bv[:])
        nc.sync.dma_start(eye_sb[:], eye[:])
        load_xt_block(1)
        nc.sync.dma_start(tri2_sb[:], tri2[:])
        load_xt_block(2)
        load_xt_block(3)
        nc.vector.memset(ones_sb[:], 1.0)
        for blk8 in range(4, 8):
            load_xt_block(blk8)
        nc.sync.dma_start(bo_sb[:], bo[None, :])
        nc.sync.dma_start(
            wo_sb[:].rearrange("p (c n) -> p c n", c=8),
            wo.rearrange("(c p) n -> p c n", p=128),
        )

        # Q^T | K^T packed: col t -> Q^T, col TI + t -> K^T  (channels on partitions)
        qkt_sb = qkvp.tile([128, 2 * TI], BF16)
        # V' : [kpos(128), 32 ktiles x (2 heads x 65)]; col 130*kt + 65*h + d,
        # d==64 is the ones column (softmax denominator trick)
        vp_sb = qkvp.tile([128, 32 * 130], BF16)
        vp_ones = vp_sb.rearrange("p (kt h d) -> p kt h d", kt=32, h=2, d=65)[
            :, :, :, 64:65
        ]
        nc.vector.memset(vp_ones, 1.0)

        # two half-sized AllToAlls: b=0 slices, then b=1 slices.
        # rows 256*j + 128*sl + 64*h + p  (slice s=2j+sl -> dest core s)
        a2a_in = [dram.tile([1024, 256], BF16, name=f"a2a_in{b}") for b in range(2)]
        a2a_out = [dram.tile([1024, 256], BF16, name=f"a2a_out{b}") for b in range(2)]

        # ---- phase A/B: QKV projections, per 512-token block.
        # Generator of PE-sized chunks so batch 1's projection work can be
        # interleaved into batch 0's attention emission.
        def ab_block(b, blk):
            base = 2048 * b + 512 * blk

            def proj(w_sb, b_sb, dst):
                ps = psS.tile([128, 512], F32, name="ps_p", tag="pss")
                for c in range(8):
                    nc.tensor.matmul(
                        ps[:],
                        w_sb[:, 128 * c : 128 * (c + 1)],
                        xt_sb[:, 4096 * c + base : 4096 * c + base + 512],
                        start=(c == 0),
                        stop=(c == 7),
                    )
                # evacuate with bias folded in (per-partition scalar add)
                nc.vector.tensor_scalar_add(dst, ps[:], b_sb[:])

            proj(wq_sb, bq_sb, qkt_sb[:, base : base + 512])
            yield
            proj(wk_sb, bk_sb, qkt_sb[:, TI + base : TI + base + 512])
            yield
            vt_blk = vtb.tile([128, 512], BF16, name="vt_blk")
            proj(wv_sb, bv_sb, vt_blk[:])
            yield
            # V' tiles via PE transpose
            ps2 = psS.tile([128, 512], BF16, name="ps_vt", tag="pss")
            for i in range(4):
                nc.tensor.transpose(
                    ps2[:, 128 * i : 128 * (i + 1)],
                    vt_blk[:, 128 * i : 128 * (i + 1)],
                    eye_sb[:],
                )
            kt0 = 16 * b + 4 * blk
            dst = vp_sb[:, 130 * kt0 : 130 * (kt0 + 4)].rearrange(
                "p (kt h d) -> p kt h d", kt=4, h=2, d=65
            )[:, :, :, :64]
            src = ps2[:].rearrange("p (i h d) -> p i h d", i=4, h=2, d=64)
            nc.vector.tensor_copy(dst, src)
            yield

        # batch 0 projections emitted up front; batch 1 paced into phase C
        for blk in range(4):
            for _ in ab_block(0, blk):
                pass
        ab1 = itertools.chain.from_iterable(ab_block(1, blk) for blk in range(4))

        # ---- out-projection pass for one 256-token half (after a2a b) ------------
        def outproj_pass(b, potag="po", pobufs=1):
            attn2 = aoutp.tile([128, 8 * 256], BF16, name="attn2")  # col 256c+t
            for c in range(8):
                nc.sync.dma_start(
                    attn2[:, 256 * c : 256 * (c + 1)],
                    a2a_out[b][128 * c : 128 * (c + 1), :],
                )
            for mt in range(2):
                for nh in range(2):
                    sl = slice(512 * nh, 512 * (nh + 1))
                    po = psB.tile([128, 512], F32, name="ps_o", tag=potag, bufs=pobufs)
                    for c in range(8):
                        nc.tensor.matmul(
                            po[:],
                            attn2[:, 256 * c + 128 * mt : 256 * c + 128 * (mt + 1)],
                            wo_sb[:, 1024 * c + 512 * nh : 1024 * c + 512 * (nh + 1)],
                            start=(c == 0),
                            stop=False,
                        )
                    nc.tensor.matmul(
                        po[:], ones_sb[:, 0:128], bo_sb[:, sl], start=False, stop=True
                    )
                    o_t = osb.tile([128, 512], F32, name="o_t")
                    nc.vector.tensor_copy(o_t[:], po[:])
                    nc.sync.dma_start(
                        out[256 * b + 128 * mt : 256 * b + 128 * (mt + 1), sl], o_t[:]
                    )
                    yield

        # ---- phase C: attention; two (batch, q-block) groups in flight -----------
        class Group:
            def __init__(self, b, j):
                self.b, self.j = b, j
                self.nkb = 4 * j + 4
                self.kb_s = 0  # next k-block to score
                self.kb_a = 0  # next k-block to accumulate into AV
                self.qbase = 2048 * b + 512 * j
                self.avs = [
                    psB.tile([128, 512], F32, name=f"av{h}", tag="av")
                    for h in range(2)
                ]
                self.pts = {}

            def emit_scores(self):
                kb = self.kb_s
                self.kb_s += 1
                m = kb - 4 * self.j
                off = 128 * m if m >= 0 else 0
                kbase = TI + 2048 * self.b + 128 * kb
                pt = ptp.tile([128, 1024], BF16, name="pt")
                for h in range(2):
                    hr = 64 * h
                    ps_s = psS.tile([128, 512], F32, name="ps_s", tag="pss")
                    nc.tensor.matmul(
                        ps_s[:, off:512],
                        qkt_sb[hr : hr + 64, kbase : kbase + 128],
                        qkt_sb[hr : hr + 64, self.qbase + off : self.qbase + 512],
                        start=True,
                        stop=True,
                    )
                    nc.scalar.activation(
                        pt[:, 512 * h + off : 512 * (h + 1)],
                        ps_s[:, off:512],
                        mybir.ActivationFunctionType.Exp,
                        scale=SCALE,
                    )
                if m >= 0:
                    # zero the strictly-upper triangle of the diagonal
                    # 128x128 tile (both heads in one DVE op)
                    ptd = pt.rearrange("p (s t) -> p s t", s=2)[:, :, off : off + 128]
                    nc.vector.tensor_mul(
                        ptd, ptd, tri2_sb[:].rearrange("p (s t) -> p s t", s=2)
                    )
                self.pts[kb] = (pt, off)

            def emit_av(self):
                kb = self.kb_a
                self.kb_a += 1
                pt, off = self.pts.pop(kb)
                vb = 130 * (16 * self.b + kb)
                for h in range(2):
                    nc.tensor.matmul(
                        self.avs[h][0:65, off:],
                        vp_sb[:, vb + 65 * h : vb + 65 * h + 65],
                        pt[:, 512 * h + off : 512 * (h + 1)],
                        start=(kb == 0),
                        stop=(kb == self.nkb - 1),
                    )

            def finalize(self):
                for h in range(2):
                    # custom-DVE ops drop the input partition offset: stage the
                    # den row to a partition-0 SBUF tile before the reciprocal
                    den = rp.tile([1, 512], F32, name="den")
                    nc.vector.tensor_copy(den[:], self.avs[h][64:65, :])
                    rec = rp.tile([1, 512], F32, name="rec")
                    nc.vector.reciprocal_approx_fast(rec[:], den[:])
                    rbc = rp.tile([64, 512], F32, name="rbc")
                    nc.gpsimd.partition_broadcast(rbc[:], rec[:])
                    at = atp.tile([64, 512], BF16, name="at")
                    nc.vector.tensor_mul(at[:], self.avs[h][0:64, :], rbc[:])
                    dst = a2a_in[self.b].rearrange(
                        "(j sl h p) q -> j h p sl q", j=4, sl=2, h=2, p=64
                    )[self.j, h]
                    nc.sync.dma_start(
                        dst, at[:].rearrange("p (sl q) -> p sl q", sl=2)
                    )

        def emit_a2a(b):
            nc.gpsimd.collective_compute(
                "AllToAll",
                mybir.AluOpType.bypass,
                replica_groups=[list(range(NC))],
                ins=[a2a_in[b].opt()],
                outs=[a2a_out[b].opt()],
            )

        # long groups first so two groups stay in flight most of the time
        queue = [(0, 3), (0, 2), (0, 1), (0, 0), (1, 3), (1, 2), (1, 1), (1, 0)]
        active = []
        state = {
            "ab1_done": False,
            "b0_left": 4,
            "b1_done": 0,
            "op0": None,
            "op0_emitted": 0,
        }

        def pace_ab1(n=1):
            if state["ab1_done"]:
                return
            for _ in range(n):
                if next(ab1, "end") == "end":
                    state["ab1_done"] = True
                    return

        def pace_op0(n=1, cap=2):
            # cap: keep some pass-0 chunks in reserve to fill the PE while the
            # second AllToAll runs
            if state["op0"] is None:
                return
            for _ in range(n):
                if state["op0_emitted"] >= cap:
                    return
                if next(state["op0"], "end") == "end":
                    return
                state["op0_emitted"] += 1

        while queue or active:
            while len(active) < 2 and queue:
                if queue[0][0] == 1:
                    pace_ab1(1000)  # batch-1 group: its inputs must be emitted
                g = Group(*queue.pop(0))
                g.emit_scores()
                active.append(g)
            for g in list(active):
                if g.kb_s < g.nkb:
                    g.emit_scores()
                pace_ab1(1)
                pace_op0(1)
                # AV trails scores by 2 k-blocks so the exp always has slack
                # before the PE consumes it (drain once scores are exhausted)
                if g.kb_a < g.kb_s - 1 or (g.kb_s == g.nkb and g.kb_a < g.nkb):
                    g.emit_av()
                if g.kb_a == g.nkb:
                    g.finalize()
                    active.remove(g)
                    if g.b == 0:
                        state["b0_left"] -= 1
                        if state["b0_left"] == 0:
                            emit_a2a(0)  # overlaps batch-1 attention
                    else:
                        state["b1_done"] += 1
                        if state["b1_done"] == 2 and state["op0"] is None:
                            # second b1 group done: a2a#1 long finished; start
                            # the b0-half output projection in the remaining
                            # attention window
                            state["op0"] = outproj_pass(0)
        pace_ab1(1000)

        # ---- tail: second reshard; remaining pass-0 chunks run during it ---------
        emit_a2a(1)
        if state["op0"] is None:
            state["op0"] = outproj_pass(0)
        pace_op0(1000, cap=1000)
        for _ in outproj_pass(1, potag="av", pobufs=4):
            pass

    nc.compile()
    return nc


_NC_CACHE = None


def _get_nc():
    global _NC_CACHE
    if _NC_CACHE is None:
        _NC_CACHE = build_nc()
    return _NC_CACHE


def _b16(a):
    return np.ascontiguousarray(np.asarray(a, np.float32).astype(ml_dtypes.bfloat16))


def make_in_maps(x, Wq, bq, Wk, bk, Wv, bv, Wo, bo):
    xt = _b16(np.asarray(x, np.float32).reshape(TI, D).T)  # [D, TI]
    Wq, Wk, Wv, Wo = _b16(Wq), _b16(Wk), _b16(Wv), _b16(Wo)
    bo16 = _b16(bo)
    bqf = np.asarray(bq, np.float32).reshape(D, 1)
    bkf = np.asarray(bk, np.float32).reshape(D, 1)
    bvf = np.asarray(bv, np.float32).reshape(D, 1)
    # 0/1 keep mask (1 where k<=q), duplicated for both heads
    tri01 = np.where(np.arange(128)[:, None] <= np.arange(128)[None, :], 1.0, 0.0)
    tri2 = np.ascontiguousarray(
        np.concatenate([tri01, tri01], axis=1).astype(ml_dtypes.bfloat16)
    )
    eye = np.eye(128, dtype=ml_dtypes.bfloat16)
    in_maps = []
    for r in range(NC):
        ch = slice(128 * r, 128 * (r + 1))
        in_maps.append(
            {
                "xt": xt,
                "wq": np.ascontiguousarray(Wq[:, ch]),
                "wk": np.ascontiguousarray(Wk[:, ch]),
                "wv": np.ascontiguousarray(Wv[:, ch]),
                "bq": np.ascontiguousarray(bqf[ch]),
                "bk": np.ascontiguousarray(bkf[ch]),
                "bv": np.ascontiguousarray(bvf[ch]),
                "wo": Wo,
                "bo": bo16,
                "tri2": tri2,
                "eye": eye,
            }
        )
    return in_maps


def assemble(results):
    out = np.empty((B, T, D), np.float32)
    for r in range(NC):
        res = results[r]["out"]
        out[0, 256 * r : 256 * (r + 1), :] = res[0:256]
        out[1, 256 * r : 256 * (r + 1), :] = res[256:512]
    return out


def run(inputs, trace=False, **kw):
    nc = _get_nc()
    in_maps = make_in_maps(**inputs)
    res = run_bass_kernel_spmd(nc, in_maps, core_ids=list(range(NC)), trace=trace, **kw)
    return assemble(res.results), res


def kernel(**inputs) -> np.ndarray:
    out, _ = run(inputs)
    return out


# revision 6
# speedup vs baseline: 1.3515x; 1.0395x over previous
"""Causal self-attention (B=2, T=2048, D=1024, H=16) on 8 TRN2 NeuronCores.

Sharding: 8-way tensor-parallel over heads (2 heads/core, both batches).
The head->token reshard is split into TWO AllToAlls (batch 0, then batch 1)
so the first collective and half the output projection overlap with batch-1
attention compute; slice ownership is 256-token interleaved (core r owns
tokens [256r, 256r+256) of BOTH batches).

Per-core program (SPMD, identical program, per-core data):
  core r: heads {2r, 2r+1}  -> qkv channel slice [128r : 128r+128)
          output slice      -> tokens [256r, 256r+256) of b0 and b1

x^T is pre-transposed on the host (removes all PE transposes of x), QKV
biases are folded into the PSUM evacuation (DVE tensor_scalar_add with a
per-partition bias), the causal mask is a 0/1 DVE multiply on the exp'd
probabilities (no mask matmuls), and softmax normalization uses
reciprocal_approx_fast (~5x faster than DVE reciprocal, ~18 bits).

bf16 matmul inputs, fp32 PSUM accumulation, fp32 output.

Attention is processed as (batch, 512-query-block) groups, two groups in
flight round-robin; per k-block each head's scores go to a 1-bank PSUM tile
(pss tag, 3 bufs) and one Exp per head on ScalarE. Causality is handled by
skipping above-diagonal k-blocks plus the 0/1 mask mul on the diagonal tile.
The softmax denominator comes from an appended ones-column in V';
normalization is reciprocal + GpSimd partition-broadcast.

PSUM budget: pss 3x1 bank + av (attention accumulators) 4x1 bank +
po (out-proj, dedicated tag to avoid PE-FIFO/slot-release deadlocks) 1 bank.
"""

import numpy as np
import ml_dtypes
import itertools
from contextlib import ExitStack

import concourse.bass as bass
import concourse.tile as tile
from concourse import mybir, bacc
from concourse.bass_utils import run_bass_kernel_spmd

F32 = mybir.dt.float32
BF16 = mybir.dt.bfloat16

B, T, D, H, HD = 2, 2048, 1024, 16, 64
NC = 8  # cores
TI = B * T  # token instances = 4096
SCALE = HD ** -0.5


def build_nc() -> bass.Bass:
    nc = bacc.Bacc("TRN2", target_bir_lowering=False, debug=False, num_devices=NC)

    # x^T host-packed per 512-token block: xt[g][p, 512c+t] = x[512g+t, 128c+p]
    xt = nc.dram_tensor("xt", [8, 128, TI], BF16, kind="ExternalInput").ap()
    wq = nc.dram_tensor("wq", [D, 128], BF16, kind="ExternalInput").ap()
    wk = nc.dram_tensor("wk", [D, 128], BF16, kind="ExternalInput").ap()
    wv = nc.dram_tensor("wv", [D, 128], BF16, kind="ExternalInput").ap()
    bq = nc.dram_tensor("bq", [128, 1], F32, kind="ExternalInput").ap()
    bk = nc.dram_tensor("bk", [128, 1], F32, kind="ExternalInput").ap()
    bv = nc.dram_tensor("bv", [128, 1], F32, kind="ExternalInput").ap()
    wo = nc.dram_tensor("wo", [D, D], BF16, kind="ExternalInput").ap()
    bo = nc.dram_tensor("bo", [D], BF16, kind="ExternalInput").ap()
    # 0/1 causal keep-mask (1 where k<=q), duplicated horizontally for 2 heads
    tri2 = nc.dram_tensor("tri2", [128, 256], BF16, kind="ExternalInput").ap()
    eye = nc.dram_tensor("eye", [128, 128], BF16, kind="ExternalInput").ap()
    out = nc.dram_tensor("out", [512, D], F32, kind="ExternalOutput").ap()

    with tile.TileContext(nc) as tc, ExitStack() as ctx:
        const = ctx.enter_context(tc.tile_pool(name="const", bufs=1))
        qkvp = ctx.enter_context(tc.tile_pool(name="qkvp", bufs=1))
        xtp = ctx.enter_context(tc.tile_pool(name="xtp", bufs=1))
        vtb = ctx.enter_context(tc.tile_pool(name="vtb", bufs=2))
        ptp = ctx.enter_context(tc.tile_pool(name="ptp", bufs=6))
        rp = ctx.enter_context(tc.tile_pool(name="rp", bufs=2))
        atp = ctx.enter_context(tc.tile_pool(name="atp", bufs=3))
        aoutp = ctx.enter_context(tc.tile_pool(name="aoutp", bufs=2))
        osb = ctx.enter_context(tc.tile_pool(name="osb", bufs=2))
        psS = ctx.enter_context(tc.tile_pool(name="psS", bufs=3, space="PSUM"))
        psB = ctx.enter_context(tc.tile_pool(name="psB", bufs=4, space="PSUM"))
        dram = ctx.enter_context(tc.tile_pool(name="dram", bufs=1, space="DRAM"))

        # ---- constants / weights -------------------------------------------------
        # DMA order matters: the PE's first work (batch-0 projections) needs
        # wq/wk/wv + xt blocks 0-3; everything else (wo especially, 2MB) waits.
        wq_sb = const.tile([128, D], BF16)  # col 128c+m  <- wq[128c+p, m]
        wk_sb = const.tile([128, D], BF16)
        wv_sb = const.tile([128, D], BF16)
        bq_sb = const.tile([128, 1], F32)
        bk_sb = const.tile([128, 1], F32)
        bv_sb = const.tile([128, 1], F32)
        bo_sb = const.tile([1, D], BF16)
        wo_sb = const.tile([128, 8 * D], BF16)  # col 1024c+n <- wo[128c+p, n]
        tri2_sb = const.tile([128, 256], BF16)
        eye_sb = const.tile([128, 128], BF16)
        ones_sb = const.tile([1, 512], BF16)
        xt_sb = xtp.tile([128, 8 * TI], BF16)  # col 4096g + 512c + t (g=token block)

        def load_xt_block(blk8):
            nc.sync.dma_start(
                xt_sb[:, TI * blk8 : TI * (blk8 + 1)], xt[blk8]
            )

        for w_sb, w in ((wq_sb, wq), (wk_sb, wk), (wv_sb, wv)):
            nc.sync.dma_start(
                w_sb[:].rearrange("p (c m) -> p c m", c=8),
                w.rearrange("(c p) m -> p c m", p=128),
            )
        load_xt_block(0)
        nc.sync.dma_start(bq_sb[:], bq[:])
        nc.sync.dma_start(bk_sb[:], bk[:])
        nc.sync.dma_start(bv_sb[:], bv[:])
        nc.sync.dma_start(eye_sb[:], eye[:])
        load_xt_block(1)
        nc.sync.dma_start(tri2_sb[:], tri2[:])
        load_xt_block(2)
        load_xt_block(3)
        nc.vector.memset(ones_sb[:], 1.0)
        for blk8 in range(4, 8):
            load_xt_block(blk8)
        nc.sync.dma_start(bo_sb[:], bo[None, :])
        nc.sync.dma_start(
            wo_sb[:].rearrange("p (c n) -> p c n", c=8),
            wo.rearrange("(c p) n -> p c n", p=128),
        )

        # Q^T | K^T packed: col t -> Q^T, col TI + t -> K^T  (channels on partitions)
        qkt_sb = qkvp.tile([128, 2 * TI], BF16)
        # V' : [kpos(128), 32 ktiles x (2 heads x 128)]; col 256*kt + 128*h + d.
        # d 0:64 are ones columns: the AV matmul then emits the softmax
        # denominator already replicated on partitions 0:64 (no partition
        # broadcast needed, keeps the Pool queue free for the collectives);
        # d 64:128 are the V values.
        vp_sb = qkvp.tile([128, 32 * 256], BF16)
        vp_ones = vp_sb.rearrange("p (kt h d) -> p kt h d", kt=32, h=2, d=128)[
            :, :, :, 0:64
        ]
        nc.vector.memset(vp_ones, 1.0)

        # two half-sized AllToAlls: b=0 slices, then b=1 slices.
        # rows 256*j + 128*sl + 64*h + p  (slice s=2j+sl -> dest core s)
        a2a_in = [dram.tile([1024, 256], BF16, name=f"a2a_in{b}") for b in range(2)]
        a2a_out = [dram.tile([1024, 256], BF16, name=f"a2a_out{b}") for b in range(2)]

        # ---- phase A/B: QKV projections, per 512-token block.
        # Generator of PE-sized chunks so batch 1's projection work can be
        # interleaved into batch 0's attention emission.
        def ab_block(b, blk):
            base = 2048 * b + 512 * blk
            g = 4 * b + blk

            def proj(w_sb, b_sb, dst):
                ps = psS.tile([128, 512], F32, name="ps_p", tag="pss")
                for c in range(8):
                    nc.tensor.matmul(
                        ps[:],
                        w_sb[:, 128 * c : 128 * (c + 1)],
                        xt_sb[:, 4096 * g + 512 * c : 4096 * g + 512 * (c + 1)],
                        start=(c == 0),
                        stop=(c == 7),
                    )
                # evacuate with bias folded in (per-partition scalar add)
                nc.vector.tensor_scalar_add(dst, ps[:], b_sb[:])

            proj(wq_sb, bq_sb, qkt_sb[:, base : base + 512])
            yield
            proj(wk_sb, bk_sb, qkt_sb[:, TI + base : TI + base + 512])
            yield
            vt_blk = vtb.tile([128, 512], BF16, name="vt_blk")
            proj(wv_sb, bv_sb, vt_blk[:])
            yield
            # V' tiles via PE transpose
            ps2 = psS.tile([128, 512], BF16, name="ps_vt", tag="pss")
            for i in range(4):
                nc.tensor.transpose(
                    ps2[:, 128 * i : 128 * (i + 1)],
                    vt_blk[:, 128 * i : 128 * (i + 1)],
                    eye_sb[:],
                )
            kt0 = 16 * b + 4 * blk
            dst = vp_sb[:, 256 * kt0 : 256 * (kt0 + 4)].rearrange(
                "p (kt h d) -> p kt h d", kt=4, h=2, d=128
            )[:, :, :, 64:128]
            src = ps2[:].rearrange("p (i h d) -> p i h d", i=4, h=2, d=64)
            nc.vector.tensor_copy(dst, src)
            yield

        # batch 0 projections emitted up front; batch 1 paced into phase C
        for blk in range(4):
            for _ in ab_block(0, blk):
                pass
        ab1 = itertools.chain.from_iterable(ab_block(1, blk) for blk in range(4))

        # ---- out-projection pass for one 256-token half (after a2a b) ------------
        def outproj_pass(b, potag="po", pobufs=1):
            attn2 = aoutp.tile([128, 8 * 256], BF16, name="attn2")  # col 256c+t
            for c in range(8):
                nc.sync.dma_start(
                    attn2[:, 256 * c : 256 * (c + 1)],
                    a2a_out[b][128 * c : 128 * (c + 1), :],
                )
            for mt in range(2):
                for nh in range(2):
                    sl = slice(512 * nh, 512 * (nh + 1))
                    po = psB.tile([128, 512], F32, name="ps_o", tag=potag, bufs=pobufs)
                    for c in range(8):
                        nc.tensor.matmul(
                            po[:],
                            attn2[:, 256 * c + 128 * mt : 256 * c + 128 * (mt + 1)],
                            wo_sb[:, 1024 * c + 512 * nh : 1024 * c + 512 * (nh + 1)],
                            start=(c == 0),
                            stop=False,
                        )
                    nc.tensor.matmul(
                        po[:], ones_sb[:, 0:128], bo_sb[:, sl], start=False, stop=True
                    )
                    o_t = osb.tile([128, 512], F32, name="o_t")
                    nc.vector.tensor_copy(o_t[:], po[:])
                    nc.sync.dma_start(
                        out[256 * b + 128 * mt : 256 * b + 128 * (mt + 1), sl], o_t[:]
                    )
                    yield

        # ---- phase C: attention; two (batch, q-block) groups in flight -----------
        class Group:
            def __init__(self, b, j):
                self.b, self.j = b, j
                self.nkb = 4 * j + 4
                self.kb_s = 0  # next k-block to score
                self.kb_a = 0  # next k-block to accumulate into AV
                self.qbase = 2048 * b + 512 * j
                self.avs = [
                    psB.tile([128, 512], F32, name=f"av{h}", tag="av")
                    for h in range(2)
                ]
                self.pts = {}

            def emit_scores(self):
                kb = self.kb_s
                self.kb_s += 1
                m = kb - 4 * self.j
                off = 128 * m if m >= 0 else 0
                kbase = TI + 2048 * self.b + 128 * kb
                pt = ptp.tile([128, 1024], BF16, name="pt")
                for h in range(2):
                    hr = 64 * h
                    ps_s = psS.tile([128, 512], F32, name="ps_s", tag="pss")
                    nc.tensor.matmul(
                        ps_s[:, off:512],
                        qkt_sb[hr : hr + 64, kbase : kbase + 128],
                        qkt_sb[hr : hr + 64, self.qbase + off : self.qbase + 512],
                        start=True,
                        stop=True,
                    )
                    nc.scalar.activation(
                        pt[:, 512 * h + off : 512 * (h + 1)],
                        ps_s[:, off:512],
                        mybir.ActivationFunctionType.Exp,
                        scale=SCALE,
                    )
                if m >= 0:
                    # zero the strictly-upper triangle of the diagonal
                    # 128x128 tile (both heads in one DVE op)
                    ptd = pt.rearrange("p (s t) -> p s t", s=2)[:, :, off : off + 128]
                    nc.vector.tensor_mul(
                        ptd, ptd, tri2_sb[:].rearrange("p (s t) -> p s t", s=2)
                    )
                self.pts[kb] = (pt, off)

            def emit_av(self):
                kb = self.kb_a
                self.kb_a += 1
                pt, off = self.pts.pop(kb)
                vb = 256 * (16 * self.b + kb)
                for h in range(2):
                    nc.tensor.matmul(
                        self.avs[h][:, off:],
                        vp_sb[:, vb + 128 * h : vb + 128 * h + 128],
                        pt[:, 512 * h + off : 512 * (h + 1)],
                        start=(kb == 0),
                        stop=(kb == self.nkb - 1),
                    )

            def finalize(self):
                for h in range(2):
                    # avs rows 0:64 hold the denominator replicated (ones
                    # columns of V'); base partition 0 so the custom DVE
                    # reciprocal reads the right partitions
                    rec = rp.tile([64, 512], F32, name="rec")
                    nc.vector.reciprocal_approx_fast(rec[:], self.avs[h][0:64, :])
                    at = atp.tile([64, 512], BF16, name="at")
                    nc.vector.tensor_mul(at[:], self.avs[h][64:128, :], rec[:])
                    dst = a2a_in[self.b].rearrange(
                        "(j sl h p) q -> j h p sl q", j=4, sl=2, h=2, p=64
                    )[self.j, h]
                    nc.sync.dma_start(
                        dst, at[:].rearrange("p (sl q) -> p sl q", sl=2)
                    )

        def emit_a2a(b):
            nc.gpsimd.collective_compute(
                "AllToAll",
                mybir.AluOpType.bypass,
                replica_groups=[list(range(NC))],
                ins=[a2a_in[b].opt()],
                outs=[a2a_out[b].opt()],
            )

        # long groups first so two groups stay in flight most of the time
        queue = [(0, 3), (0, 2), (0, 1), (0, 0), (1, 3), (1, 2), (1, 1), (1, 0)]
        active = []
        state = {
            "ab1_done": False,
            "b0_left": 4,
            "b1_done": 0,
            "op0": None,
            "op0_emitted": 0,
        }

        def pace_ab1(n=1):
            if state["ab1_done"]:
                return
            for _ in range(n):
                if next(ab1, "end") == "end":
                    state["ab1_done"] = True
                    return

        def pace_op0(n=1, cap=2):
            # cap: keep some pass-0 chunks in reserve to fill the PE while the
            # second AllToAll runs
            if state["op0"] is None:
                return
            for _ in range(n):
                if state["op0_emitted"] >= cap:
                    return
                if next(state["op0"], "end") == "end":
                    return
                state["op0_emitted"] += 1

        while queue or active:
            while len(active) < 2 and queue:
                if queue[0][0] == 1:
                    pace_ab1(1000)  # batch-1 group: its inputs must be emitted
                g = Group(*queue.pop(0))
                g.emit_scores()
                active.append(g)
            for g in list(active):
                if g.kb_s < g.nkb:
                    g.emit_scores()
                pace_ab1(1)
                pace_op0(1)
                # AV trails scores by 2 k-blocks so the exp always has slack
                # before the PE consumes it (drain once scores are exhausted)
                if g.kb_a < g.kb_s - 1 or (g.kb_s == g.nkb and g.kb_a < g.nkb):
                    g.emit_av()
                if g.kb_a == g.nkb:
                    g.finalize()
                    active.remove(g)
                    if g.b == 0:
                        state["b0_left"] -= 1
                        if state["b0_left"] == 0:
                            emit_a2a(0)  # overlaps batch-1 attention
                    else:
                        state["b1_done"] += 1
                        if state["b1_done"] == 2 and state["op0"] is None:
                            # second b1 group done: a2a#1 long finished; start
                            # the b0-half output projection in the remaining
                            # attention window
                            state["op0"] = outproj_pass(0)
        pace_ab1(1000)

        # ---- tail: second reshard; remaining pass-0 chunks run during it ---------
        emit_a2a(1)
        if state["op0"] is None:
            state["op0"] = outproj_pass(0)
        pace_op0(1000, cap=1000)
        for _ in outproj_pass(1, potag="av", pobufs=4):
            pass

    nc.compile()
    return nc


_NC_CACHE = None


def _get_nc():
    global _NC_CACHE
    if _NC_CACHE is None:
        _NC_CACHE = build_nc()
    return _NC_CACHE


def _b16(a):
    return np.ascontiguousarray(np.asarray(a, np.float32).astype(ml_dtypes.bfloat16))


def make_in_maps(x, Wq, bq, Wk, bk, Wv, bv, Wo, bo):
    xf = np.asarray(x, np.float32).reshape(TI, D)
    # [g, p, c, t]: xt[g][p, 512c+t] = x[512g+t, 128c+p]
    xt = _b16(
        xf.reshape(8, 512, 8, 128).transpose(0, 3, 2, 1).reshape(8, 128, TI)
    )
    Wq, Wk, Wv, Wo = _b16(Wq), _b16(Wk), _b16(Wv), _b16(Wo)
    bo16 = _b16(bo)
    bqf = np.asarray(bq, np.float32).reshape(D, 1)
    bkf = np.asarray(bk, np.float32).reshape(D, 1)
    bvf = np.asarray(bv, np.float32).reshape(D, 1)
    # 0/1 keep mask (1 where k<=q), duplicated for both heads
    tri01 = np.where(np.arange(128)[:, None] <= np.arange(128)[None, :], 1.0, 0.0)
    tri2 = np.ascontiguousarray(
        np.concatenate([tri01, tri01], axis=1).astype(ml_dtypes.bfloat16)
    )
    eye = np.eye(128, dtype=ml_dtypes.bfloat16)
    in_maps = []
    for r in range(NC):
        ch = slice(128 * r, 128 * (r + 1))
        in_maps.append(
            {
                "xt": xt,
                "wq": np.ascontiguousarray(Wq[:, ch]),
                "wk": np.ascontiguousarray(Wk[:, ch]),
                "wv": np.ascontiguousarray(Wv[:, ch]),
                "bq": np.ascontiguousarray(bqf[ch]),
                "bk": np.ascontiguousarray(bkf[ch]),
                "bv": np.ascontiguousarray(bvf[ch]),
                "wo": Wo,
                "bo": bo16,
                "tri2": tri2,
                "eye": eye,
            }
        )
    return in_maps


def assemble(results):
    out = np.empty((B, T, D), np.float32)
    for r in range(NC):
        res = results[r]["out"]
        out[0, 256 * r : 256 * (r + 1), :] = res[0:256]
        out[1, 256 * r : 256 * (r + 1), :] = res[256:512]
    return out


def run(inputs, trace=False, **kw):
    nc = _get_nc()
    in_maps = make_in_maps(**inputs)
    res = run_bass_kernel_spmd(nc, in_maps, core_ids=list(range(NC)), trace=trace, **kw)
    return assemble(res.results), res


def kernel(**inputs) -> np.ndarray:
    out, _ = run(inputs)
    return out


# revision 7
# speedup vs baseline: 1.5548x; 1.1505x over previous
"""Causal self-attention (B=2, T=2048, D=1024, H=16) on 8 TRN2 NeuronCores.

Sharding: 8-way tensor-parallel over heads (2 heads/core, both batches).
The head->token reshard is split into TWO AllToAlls (batch 0, then batch 1)
so the first collective and half the output projection overlap with batch-1
attention compute; slice ownership is 256-token interleaved (core r owns
tokens [256r, 256r+256) of BOTH batches).

Per-core program (SPMD, identical program, per-core data):
  core r: heads {2r, 2r+1}  -> qkv channel slice [128r : 128r+128)
          output slice      -> tokens [256r, 256r+256) of b0 and b1

x^T is pre-transposed on the host (removes all PE transposes of x), QKV
biases are folded into the PSUM evacuation (DVE tensor_scalar_add with a
per-partition bias), the causal mask is a 0/1 DVE multiply on the exp'd
probabilities (no mask matmuls), and softmax normalization uses
reciprocal_approx_fast (~5x faster than DVE reciprocal, ~18 bits).

bf16 matmul inputs, fp32 PSUM accumulation, fp32 output.

Attention is processed as (batch, 512-query-block) groups, two groups in
flight round-robin; per k-block each head's scores go to a 1-bank PSUM tile
(pss tag, 3 bufs) and one Exp per head on ScalarE. Causality is handled by
skipping above-diagonal k-blocks plus the 0/1 mask mul on the diagonal tile.
The softmax denominator comes from an appended ones-column in V';
normalization is reciprocal + GpSimd partition-broadcast.

PSUM budget: pss 3x1 bank + av (attention accumulators) 4x1 bank +
po (out-proj, dedicated tag to avoid PE-FIFO/slot-release deadlocks) 1 bank.
"""

import numpy as np
import ml_dtypes
import itertools
from contextlib import ExitStack

import concourse.bass as bass
import concourse.tile as tile
from concourse import mybir, bacc
from concourse.bass_utils import run_bass_kernel_spmd

F32 = mybir.dt.float32
BF16 = mybir.dt.bfloat16

B, T, D, H, HD = 2, 2048, 1024, 16, 64
NC = 8  # cores
TI = B * T  # token instances = 4096
SCALE = HD ** -0.5


def build_nc() -> bass.Bass:
    nc = bacc.Bacc("TRN2", target_bir_lowering=False, debug=False, num_devices=NC)

    # x^T host-packed per 512-token block: xt[g][p, 512c+t] = x[512g+t, 128c+p]
    xt = nc.dram_tensor("xt", [8, 128, TI], BF16, kind="ExternalInput").ap()
    # host-packed: wq[p, 128c+m] = Wq[128c+p, my_ch m]
    wq = nc.dram_tensor("wq", [128, D], BF16, kind="ExternalInput").ap()
    wk = nc.dram_tensor("wk", [128, D], BF16, kind="ExternalInput").ap()
    wv = nc.dram_tensor("wv", [128, D], BF16, kind="ExternalInput").ap()
    bq = nc.dram_tensor("bq", [128, 1], F32, kind="ExternalInput").ap()
    bk = nc.dram_tensor("bk", [128, 1], F32, kind="ExternalInput").ap()
    bv = nc.dram_tensor("bv", [128, 1], F32, kind="ExternalInput").ap()
    # host-packed: wo[p, 1024c+n] = Wo[128c+p, n]
    wo = nc.dram_tensor("wo", [128, 8 * D], BF16, kind="ExternalInput").ap()
    bo = nc.dram_tensor("bo", [D], BF16, kind="ExternalInput").ap()
    # 0/1 causal keep-mask (1 where k<=q), duplicated horizontally for 2 heads
    tri2 = nc.dram_tensor("tri2", [128, 256], BF16, kind="ExternalInput").ap()
    eye = nc.dram_tensor("eye", [128, 128], BF16, kind="ExternalInput").ap()
    out = nc.dram_tensor("out", [512, D], F32, kind="ExternalOutput").ap()

    with tile.TileContext(nc) as tc, ExitStack() as ctx:
        const = ctx.enter_context(tc.tile_pool(name="const", bufs=1))
        qkvp = ctx.enter_context(tc.tile_pool(name="qkvp", bufs=1))
        xtp = ctx.enter_context(tc.tile_pool(name="xtp", bufs=1))
        vtb = ctx.enter_context(tc.tile_pool(name="vtb", bufs=2))
        ptp = ctx.enter_context(tc.tile_pool(name="ptp", bufs=6))
        rp = ctx.enter_context(tc.tile_pool(name="rp", bufs=2))
        atp = ctx.enter_context(tc.tile_pool(name="atp", bufs=3))
        aoutp = ctx.enter_context(tc.tile_pool(name="aoutp", bufs=2))
        osb = ctx.enter_context(tc.tile_pool(name="osb", bufs=2))
        psS = ctx.enter_context(tc.tile_pool(name="psS", bufs=2, space="PSUM"))
        psB = ctx.enter_context(tc.tile_pool(name="psB", bufs=4, space="PSUM"))
        dram = ctx.enter_context(tc.tile_pool(name="dram", bufs=1, space="DRAM"))

        # ---- constants / weights -------------------------------------------------
        # DMA order matters: the PE's first work (batch-0 projections) needs
        # wq/wk/wv + xt blocks 0-3; everything else (wo especially, 2MB) waits.
        wq_sb = const.tile([128, D], BF16)  # col 128c+m  <- wq[128c+p, m]
        wk_sb = const.tile([128, D], BF16)
        wv_sb = const.tile([128, D], BF16)
        bq_sb = const.tile([128, 1], F32)
        bk_sb = const.tile([128, 1], F32)
        bv_sb = const.tile([128, 1], F32)
        bo_sb = const.tile([1, D], BF16)
        wo_sb = const.tile([128, 8 * D], BF16)  # col 1024c+n <- wo[128c+p, n]
        tri2_sb = const.tile([128, 256], BF16)
        eye_sb = const.tile([128, 128], BF16)
        ones_sb = const.tile([1, 512], BF16)
        xt_sb = xtp.tile([128, 8 * TI], BF16)  # col 4096g + 512c + t (g=token block)

        def load_xt_block(blk8):
            nc.sync.dma_start(
                xt_sb[:, TI * blk8 : TI * (blk8 + 1)], xt[blk8]
            )

        for w_sb, w in ((wq_sb, wq), (wk_sb, wk), (wv_sb, wv)):
            nc.sync.dma_start(w_sb[:], w[:])
        load_xt_block(0)
        nc.sync.dma_start(bq_sb[:], bq[:])
        nc.sync.dma_start(bk_sb[:], bk[:])
        nc.sync.dma_start(bv_sb[:], bv[:])
        nc.sync.dma_start(eye_sb[:], eye[:])
        load_xt_block(1)
        nc.sync.dma_start(tri2_sb[:], tri2[:])
        load_xt_block(2)
        load_xt_block(3)
        nc.vector.memset(ones_sb[:], 1.0)
        for blk8 in range(4, 8):
            load_xt_block(blk8)
        nc.sync.dma_start(bo_sb[:], bo[None, :])
        nc.sync.dma_start(wo_sb[:], wo[:])

        # Q^T | K^T packed: col t -> Q^T, col TI + t -> K^T  (channels on partitions)
        qkt_sb = qkvp.tile([128, 2 * TI], BF16)
        # V' : [kpos(128), 32 ktiles x (2 heads x 128)]; col 256*kt + 128*h + d.
        # d 0:64 are ones columns: the AV matmul then emits the softmax
        # denominator already replicated on partitions 0:64 (no partition
        # broadcast needed, keeps the Pool queue free for the collectives);
        # d 64:128 are the V values.
        vp_sb = qkvp.tile([128, 32 * 256], BF16)
        vp_ones = vp_sb.rearrange("p (kt h d) -> p kt h d", kt=32, h=2, d=128)[
            :, :, :, 0:64
        ]
        nc.vector.memset(vp_ones, 1.0)

        # two half-sized AllToAlls: b=0 slices, then b=1 slices.
        # rows 256*j + 128*sl + 64*h + p  (slice s=2j+sl -> dest core s)
        a2a_in = [dram.tile([1024, 256], BF16, name=f"a2a_in{b}") for b in range(2)]
        a2a_out = [dram.tile([1024, 256], BF16, name=f"a2a_out{b}") for b in range(2)]

        # ---- phase A/B: QKV projections, per 512-token block.
        # Generator of PE-sized chunks so batch 1's projection work can be
        # interleaved into batch 0's attention emission.
        def ab_block(b, blk):
            base = 2048 * b + 512 * blk
            g = 4 * b + blk

            def proj(w_sb, b_sb, dst):
                ps = psS.tile([128, 512], F32, name="ps_p", tag="pss")
                for c in range(8):
                    nc.tensor.matmul(
                        ps[:],
                        w_sb[:, 128 * c : 128 * (c + 1)],
                        xt_sb[:, 4096 * g + 512 * c : 4096 * g + 512 * (c + 1)],
                        start=(c == 0),
                        stop=(c == 7),
                    )
                # evacuate with bias folded in (per-partition scalar add)
                nc.vector.tensor_scalar_add(dst, ps[:], b_sb[:])

            proj(wq_sb, bq_sb, qkt_sb[:, base : base + 512])
            yield
            proj(wk_sb, bk_sb, qkt_sb[:, TI + base : TI + base + 512])
            yield
            vt_blk = vtb.tile([128, 512], BF16, name="vt_blk")
            proj(wv_sb, bv_sb, vt_blk[:])
            yield
            # V' tiles via PE transpose
            ps2 = psS.tile([128, 512], BF16, name="ps_vt", tag="pss")
            for i in range(4):
                nc.tensor.transpose(
                    ps2[:, 128 * i : 128 * (i + 1)],
                    vt_blk[:, 128 * i : 128 * (i + 1)],
                    eye_sb[:],
                )
            kt0 = 16 * b + 4 * blk
            dst = vp_sb[:, 256 * kt0 : 256 * (kt0 + 4)].rearrange(
                "p (kt h d) -> p kt h d", kt=4, h=2, d=128
            )[:, :, :, 64:128]
            src = ps2[:].rearrange("p (i h d) -> p i h d", i=4, h=2, d=64)
            nc.vector.tensor_copy(dst, src)
            yield

        # batch 0 projections emitted up front; batch 1 paced into phase C
        for blk in range(4):
            for _ in ab_block(0, blk):
                pass
        ab1 = itertools.chain.from_iterable(ab_block(1, blk) for blk in range(4))

        # ---- out-projection pass for one 256-token half (after a2a b) ------------
        def outproj_pass(b):
            attn2 = aoutp.tile([128, 8 * 256], BF16, name="attn2")  # col 256c+t
            for c in range(8):
                nc.sync.dma_start(
                    attn2[:, 256 * c : 256 * (c + 1)],
                    a2a_out[b][128 * c : 128 * (c + 1), :],
                )
            for mt in range(2):
                for nh in range(2):
                    sl = slice(512 * nh, 512 * (nh + 1))
                    po = psB.tile([128, 512], F32, name="ps_o", tag="av")
                    for c in range(8):
                        nc.tensor.matmul(
                            po[:],
                            attn2[:, 256 * c + 128 * mt : 256 * c + 128 * (mt + 1)],
                            wo_sb[:, 1024 * c + 512 * nh : 1024 * c + 512 * (nh + 1)],
                            start=(c == 0),
                            stop=False,
                        )
                    nc.tensor.matmul(
                        po[:], ones_sb[:, 0:128], bo_sb[:, sl], start=False, stop=True
                    )
                    o_t = osb.tile([128, 512], F32, name="o_t")
                    nc.vector.tensor_copy(o_t[:], po[:])
                    nc.sync.dma_start(
                        out[256 * b + 128 * mt : 256 * b + 128 * (mt + 1), sl], o_t[:]
                    )
                    yield

        # ---- phase C: attention; two (batch, q-block) groups in flight -----------
        class Group:
            def __init__(self, b, j):
                self.b, self.j = b, j
                self.nkb = 4 * j + 4
                self.kb_s = 0  # next k-block to score
                self.kb_a = 0  # next k-block to accumulate into AV
                self.qbase = 2048 * b + 512 * j
                self.avs = [
                    psB.tile([128, 512], F32, name=f"av{h}", tag="av")
                    for h in range(2)
                ]
                self.pts = {}

            def emit_scores(self):
                kb = self.kb_s
                self.kb_s += 1
                m = kb - 4 * self.j
                off = 128 * m if m >= 0 else 0
                kbase = TI + 2048 * self.b + 128 * kb
                pt = ptp.tile([128, 1024], BF16, name="pt")
                ps_s = psS.tile([128, 1024], F32, name="ps_s", tag="pss")
                for h in range(2):
                    hr = 64 * h
                    nc.tensor.matmul(
                        ps_s[:, 512 * h + off : 512 * (h + 1)],
                        qkt_sb[hr : hr + 64, kbase : kbase + 128],
                        qkt_sb[hr : hr + 64, self.qbase + off : self.qbase + 512],
                        start=True,
                        stop=True,
                    )
                # one Exp covers both heads (strided view over the 2 banks)
                pt_v = pt.rearrange("p (s t) -> p s t", s=2)[:, :, off:512]
                ps_v = ps_s.rearrange("p (s t) -> p s t", s=2)[:, :, off:512]
                nc.scalar.activation(
                    pt_v, ps_v, mybir.ActivationFunctionType.Exp, scale=SCALE
                )
                if m >= 0:
                    # zero the strictly-upper triangle of the diagonal
                    # 128x128 tile (both heads in one DVE op)
                    ptd = pt.rearrange("p (s t) -> p s t", s=2)[:, :, off : off + 128]
                    nc.vector.tensor_mul(
                        ptd, ptd, tri2_sb[:].rearrange("p (s t) -> p s t", s=2)
                    )
                self.pts[kb] = (pt, off)

            def emit_av(self):
                kb = self.kb_a
                self.kb_a += 1
                pt, off = self.pts.pop(kb)
                vb = 256 * (16 * self.b + kb)
                for h in range(2):
                    nc.tensor.matmul(
                        self.avs[h][:, off:],
                        vp_sb[:, vb + 128 * h : vb + 128 * h + 128],
                        pt[:, 512 * h + off : 512 * (h + 1)],
                        start=(kb == 0),
                        stop=(kb == self.nkb - 1),
                    )

            def finalize(self):
                for h in range(2):
                    # avs rows 0:64 hold the denominator replicated (ones
                    # columns of V'); base partition 0 so the custom DVE
                    # reciprocal reads the right partitions
                    rec = rp.tile([64, 512], F32, name="rec")
                    nc.vector.reciprocal_approx_fast(rec[:], self.avs[h][0:64, :])
                    at = atp.tile([64, 512], BF16, name="at")
                    nc.vector.tensor_mul(at[:], self.avs[h][64:128, :], rec[:])
                    dst = a2a_in[self.b].rearrange(
                        "(j sl h p) q -> j h p sl q", j=4, sl=2, h=2, p=64
                    )[self.j, h]
                    nc.sync.dma_start(
                        dst, at[:].rearrange("p (sl q) -> p sl q", sl=2)
                    )

        def emit_a2a(b):
            nc.gpsimd.collective_compute(
                "AllToAll",
                mybir.AluOpType.bypass,
                replica_groups=[list(range(NC))],
                ins=[a2a_in[b].opt()],
                outs=[a2a_out[b].opt()],
            )

        # long groups first so two groups stay in flight most of the time
        queue = [(0, 3), (0, 2), (0, 1), (0, 0), (1, 3), (1, 2), (1, 1), (1, 0)]
        active = []
        state = {"ab1_done": False, "b0_left": 4}

        def pace_ab1(n=1):
            if state["ab1_done"]:
                return
            for _ in range(n):
                if next(ab1, "end") == "end":
                    state["ab1_done"] = True
                    return

        while queue or active:
            while len(active) < 2 and queue:
                if queue[0][0] == 1:
                    pace_ab1(1000)  # batch-1 group: its inputs must be emitted
                g = Group(*queue.pop(0))
                g.emit_scores()
                active.append(g)
            for g in list(active):
                if g.kb_s < g.nkb:
                    g.emit_scores()
                pace_ab1(1)
                # AV trails scores by 2 k-blocks so the exp always has slack
                # before the PE consumes it (drain once scores are exhausted)
                if g.kb_a < g.kb_s - 1 or (g.kb_s == g.nkb and g.kb_a < g.nkb):
                    g.emit_av()
                if g.kb_a == g.nkb:
                    g.finalize()
                    active.remove(g)
                    if g.b == 0:
                        state["b0_left"] -= 1
                        if state["b0_left"] == 0:
                            emit_a2a(0)  # overlaps batch-1 attention
        pace_ab1(1000)

        # ---- tail: pass-0 out-proj fills the second reshard's window -------------
        for _ in outproj_pass(0):
            pass
        emit_a2a(1)
        for _ in outproj_pass(1):
            pass

    nc.compile()
    return nc


_NC_CACHE = None


def _get_nc():
    global _NC_CACHE
    if _NC_CACHE is None:
        _NC_CACHE = build_nc()
    return _NC_CACHE


def _b16(a):
    return np.ascontiguousarray(np.asarray(a, np.float32).astype(ml_dtypes.bfloat16))


def make_in_maps(x, Wq, bq, Wk, bk, Wv, bv, Wo, bo):
    xf = np.asarray(x, np.float32).reshape(TI, D)
    # [g, p, c, t]: xt[g][p, 512c+t] = x[512g+t, 128c+p]
    xt = _b16(
        xf.reshape(8, 512, 8, 128).transpose(0, 3, 2, 1).reshape(8, 128, TI)
    )
    Wq, Wk, Wv, Wo = _b16(Wq), _b16(Wk), _b16(Wv), _b16(Wo)
    bo16 = _b16(bo)
    bqf = np.asarray(bq, np.float32).reshape(D, 1)
    bkf = np.asarray(bk, np.float32).reshape(D, 1)
    bvf = np.asarray(bv, np.float32).reshape(D, 1)
    # 0/1 keep mask (1 where k<=q), duplicated for both heads
    tri01 = np.where(np.arange(128)[:, None] <= np.arange(128)[None, :], 1.0, 0.0)
    tri2 = np.ascontiguousarray(
        np.concatenate([tri01, tri01], axis=1).astype(ml_dtypes.bfloat16)
    )
    eye = np.eye(128, dtype=ml_dtypes.bfloat16)
    def pack_w(W):  # [1024, 128] -> [128, 1024]: out[p, 128c+m] = W[128c+p, m]
        return np.ascontiguousarray(
            W.reshape(8, 128, 128).transpose(1, 0, 2).reshape(128, 1024)
        )

    wo_p = np.ascontiguousarray(  # [128, 8192]: out[p, 1024c+n] = Wo[128c+p, n]
        Wo.reshape(8, 128, 1024).transpose(1, 0, 2).reshape(128, 8192)
    )
    in_maps = []
    for r in range(NC):
        ch = slice(128 * r, 128 * (r + 1))
        in_maps.append(
            {
                "xt": xt,
                "wq": pack_w(Wq[:, ch]),
                "wk": pack_w(Wk[:, ch]),
                "wv": pack_w(Wv[:, ch]),
                "bq": np.ascontiguousarray(bqf[ch]),
                "bk": np.ascontiguousarray(bkf[ch]),
                "bv": np.ascontiguousarray(bvf[ch]),
                "wo": wo_p,
                "bo": bo16,
                "tri2": tri2,
                "eye": eye,
            }
        )
    return in_maps


def assemble(results):
    out = np.empty((B, T, D), np.float32)
    for r in range(NC):
        res = results[r]["out"]
        out[0, 256 * r : 256 * (r + 1), :] = res[0:256]
        out[1, 256 * r : 256 * (r + 1), :] = res[256:512]
    return out


def run(inputs, trace=False, **kw):
    nc = _get_nc()
    in_maps = make_in_maps(**inputs)
    res = run_bass_kernel_spmd(nc, in_maps, core_ids=list(range(NC)), trace=trace, **kw)
    return assemble(res.results), res


def kernel(**inputs) -> np.ndarray:
    out, _ = run(inputs)
    return out
